# revision 7
# baseline (speedup 1.0000x reference)
"""Trainium2 Bass kernel for nn_DennisNode (T=1024, N=65536, 8 cores).

The per-node recurrence (carry Z, X1, E; health/phase in the reference are
dead code -- they never feed the output):
    tension = |zn_t - Z|
    E'  = min(max(0.98 E + 100 tension, 0), 1e6)
    X1' = (X1 + if + 0.005 E') - cp*(X1 + if + 0.005 E')
    phi = 0.5 (X1' + (X1' - ir)); X3 = phi (1-dm); Y = |X3 - phi|
    Z'  = X3*Y + (tension<0.01 ? -cb : cb)*0.1 + na*nz_t

Nodes are independent -> shard N across the 8 cores (column slices). Z feeds
the next step only through |zn - Z| into the clipped E and the coupling
compare, so once E saturates at 1e6 on every node and X1 (a contraction,
ratio 1-cp per step) collapses to one shared fp32 value, the per-node state
is a shared scalar orbit and Z_t = cc_t + na*nz_t elementwise. The kernel:
  1. runs an exact serial head chunk (TC steps) on device, nodes 8-way,
     ~16 DVE instructions per step (scalar_tensor_tensor fusions that are
     bit-identical to the unfused op order);
  2. verifies the absorbing state on host with exact fp32 bounds;
  3. fast path: computes the tail with one fused tensor_scalar per
     [128 x nodes] tile (DMA-bound, ~roofline);
  4. else: falls back to more serial head chunks (always correct).
"""
import sys, os
sys.path.insert(0, "/opt/trn_rl_repo")

import numpy as np

import concourse.bass as bass
import concourse.mybir as mybir
from concourse.tile import TileContext
from concourse.bass_utils import run_bass_kernel_spmd

F32 = mybir.dt.float32
Alu = mybir.AluOpType
Ax = mybir.AxisListType

T, N = 1024, 65536
NCORES = 8
NL = N // NCORES          # nodes per core
P = 128                   # partitions
FDN = NL // P             # free-dim nodes per core
TB = 32                   # steps per DMA block in head
TC = 192                 # head-chunk steps (X1 merge happens at step 190)
SAT_E = np.float32(1e6)

# ---------------------------------------------------------------- walrus fix
_ctr = [0]


def _fix_sync_waits(nc, max_waits: int = 1):
    """This walrus build rejects >1 semaphore wait per instruction
    (CoreV3 setupSyncWait). Hoist excess waits onto same-engine NOPs."""
    for fn in nc.m.functions:
        for blk in fn.blocks:
            out, changed = [], False
            for ins in blk.instructions:
                si = ins.sync_info
                if si is not None and len(si.on_wait) > max_waits:
                    waits = list(si.on_wait)
                    head, tail = waits[:-max_waits], waits[-max_waits:]
                    for j in range(0, len(head), max_waits):
                        _ctr[0] += 1
                        nop = mybir.InstNoOp(
                            name=f"I-waitsplit-{_ctr[0]}",
                            engine=ins.engine,
                            bass_nofuse=True,
                            sync_info=mybir.SyncInfo(
                                on_wait=head[j : j + max_waits], on_update=[]
                            ),
                        )
                        nc.register_instruction(nop, overwrite=True)
                        out.append(nop)
                    ins.sync_info = mybir.SyncInfo(
                        on_wait=tail, on_update=list(si.on_update)
                    )
                    changed = True
                out.append(ins)
            if changed:
                blk.instructions = out


# ---------------------------------------------------------------- head kernel
def _build_head(scal):
    """Serial chunk kernel: TC steps over this core's NL nodes.

    zn/nz [TC, NL], state_in [3, P, FDN] (Z, X1, E), zout [TC, NL],
    state_out likewise. Each step is 16 DVE instructions, every one
    bit-identical to the plain unfused fp32 op order of the reference
    (scalar_tensor_tensor computes fl(fl(in0 op0 s) op1 in1); negation and
    the +-c01 select arithmetic are exact).
    """
    cb, iff, ir, cp, dm, na = (np.float32(scal[k]) for k in (
        "coupling_base", "internal_forward", "internal_reverse",
        "center_pull", "damping", "noise_amplitude"))
    c01 = np.float32(cb * np.float32(0.1))
    dm1 = np.float32(np.float32(1.0) - dm)

    nc = bass.Bass()
    zn = nc.dram_tensor("zn", [TC, NL], F32, kind="ExternalInput")
    nz = nc.dram_tensor("nz", [TC, NL], F32, kind="ExternalInput")
    st_in = nc.dram_tensor("state_in", [3, P, FDN], F32, kind="ExternalInput")
    zout = nc.dram_tensor("zout", [TC, NL], F32, kind="ExternalOutput")
    st_out = nc.dram_tensor("state_out", [3, P, FDN], F32, kind="ExternalOutput")

    zn_b = zn.rearrange("(b t) (p f) -> b p t f", t=TB, p=P)
    nz_b = nz.rearrange("(b t) (p f) -> b p t f", t=TB, p=P)
    zo_b = zout.rearrange("(b t) (p f) -> b p t f", t=TB, p=P)
    nblocks = TC // TB

    with TileContext(nc) as tc:
        with (
            tc.tile_pool(name="st", bufs=1) as stp,
            tc.tile_pool(name="io", bufs=2) as iop,
            tc.tile_pool(name="wk", bufs=2) as wkp,
        ):
            Zs = stp.tile([P, FDN], F32, name="Zs")
            X1 = stp.tile([P, FDN], F32, name="X1")
            Es = stp.tile([P, FDN], F32, name="Es")
            nc.sync.dma_start(out=Zs[:], in_=st_in[0])
            nc.sync.dma_start(out=X1[:], in_=st_in[1])
            nc.sync.dma_start(out=Es[:], in_=st_in[2])

            V = nc.vector
            i = [0]

            def step(zn_s, nz_s, zprev):
                w = {}
                for nm in ("d", "ten", "e2", "e3", "aa", "x1b", "s",
                           "phi", "yd", "Y", "p", "m2", "cc"):
                    w[nm] = wkp.tile([P, FDN], F32, name=f"{nm}_{i[0]}", tag=nm)[:]
                i[0] += 1
                V.tensor_tensor(out=w["d"], in0=zn_s, in1=zprev, op=Alu.subtract)
                d3 = w["d"].rearrange("p (f o) -> p f o", o=1)
                t3 = w["ten"].rearrange("p (f o) -> p f o", o=1)
                V.tensor_reduce(out=t3, in_=d3, axis=Ax.X, op=Alu.max,
                                apply_absolute_value=True)
                V.tensor_scalar(out=w["e2"], in0=w["ten"], scalar1=100.0,
                                scalar2=None, op0=Alu.mult)
                V.scalar_tensor_tensor(out=w["e3"], in0=Es[:], scalar=0.98,
                                       in1=w["e2"], op0=Alu.mult, op1=Alu.add)
                # e3 >= 0 always (0.98E >= 0, 100|d| >= 0), so the max(.,0)
                # side of the reference clip is an exact identity.
                V.tensor_scalar(out=Es[:], in0=w["e3"], scalar1=1e6,
                                scalar2=None, op0=Alu.min)
                V.tensor_scalar(out=w["aa"], in0=Es[:], scalar1=0.005,
                                scalar2=None, op0=Alu.mult)
                V.scalar_tensor_tensor(out=w["x1b"], in0=X1[:], scalar=float(iff),
                                       in1=w["aa"], op0=Alu.add, op1=Alu.add)
                V.scalar_tensor_tensor(out=X1[:], in0=w["x1b"], scalar=float(-cp),
                                       in1=w["x1b"], op0=Alu.mult, op1=Alu.add)
                V.scalar_tensor_tensor(out=w["s"], in0=X1[:], scalar=float(-ir),
                                       in1=X1[:], op0=Alu.add, op1=Alu.add)
                V.tensor_scalar(out=w["phi"], in0=w["s"], scalar1=0.5,
                                scalar2=None, op0=Alu.mult)
                V.scalar_tensor_tensor(out=w["yd"], in0=w["phi"], scalar=float(dm1),
                                       in1=w["phi"], op0=Alu.mult, op1=Alu.subtract)
                y3 = w["yd"].rearrange("p (f o) -> p f o", o=1)
                Y3 = w["Y"].rearrange("p (f o) -> p f o", o=1)
                V.tensor_reduce(out=Y3, in_=y3, axis=Ax.X, op=Alu.max,
                                apply_absolute_value=True)
                V.scalar_tensor_tensor(out=w["p"], in0=w["phi"], scalar=float(dm1),
                                       in1=w["Y"], op0=Alu.mult, op1=Alu.mult)
                V.tensor_scalar(out=w["m2"], in0=w["ten"], scalar1=0.01,
                                scalar2=float(2.0 * c01), op0=Alu.is_ge, op1=Alu.mult)
                V.scalar_tensor_tensor(out=w["cc"], in0=w["m2"], scalar=float(-c01),
                                       in1=w["p"], op0=Alu.add, op1=Alu.add)
                # Z_t = fl(fl(na*nz) + cc), written over the nz slot in place
                V.scalar_tensor_tensor(out=nz_s, in0=nz_s, scalar=float(na),
                                       in1=w["cc"], op0=Alu.mult, op1=Alu.add)

            zprev = Zs[:]
            for b in range(nblocks):
                znt = iop.tile([P, TB, FDN], F32, name=f"znt{b}", tag="znt")
                nzt = iop.tile([P, TB, FDN], F32, name=f"nzt{b}", tag="nzt")
                nc.sync.dma_start(out=znt[:], in_=zn_b[b])
                nc.sync.dma_start(out=nzt[:], in_=nz_b[b])
                for s in range(TB):
                    step(znt[:, s], nzt[:, s], zprev)
                    zprev = nzt[:, s]
                nc.sync.dma_start(out=zo_b[b], in_=nzt[:])

            nc.sync.dma_start(out=st_out[0], in_=zprev)
            nc.sync.dma_start(out=st_out[1], in_=X1[:])
            nc.sync.dma_start(out=st_out[2], in_=Es[:])

    _fix_sync_waits(nc)
    return nc


# ---------------------------------------------------------------- tail kernel
def _build_tail(scal, ntail):
    """Bulk tail: Z[t, n] = fl(fl(na*nz[t,n]) + cc_t), cc per-partition."""
    na = np.float32(scal["noise_amplitude"])
    nfull, rem = divmod(ntail, P)

    nc = bass.Bass()
    nz = nc.dram_tensor("nz", [ntail, NL], F32, kind="ExternalInput")
    ccv = nc.dram_tensor("cc", [ntail, 1], F32, kind="ExternalInput")
    zout = nc.dram_tensor("zout", [ntail, NL], F32, kind="ExternalOutput")

    with TileContext(nc) as tc:
        with (
            tc.tile_pool(name="io", bufs=3) as iop,
            tc.tile_pool(name="ccp", bufs=2) as ccp,
        ):
            for b in range(nfull + (1 if rem else 0)):
                pp = P if b < nfull else rem
                t0 = b * P
                tl = iop.tile([P, NL], F32, name=f"tl{b}", tag="tl")
                cct = ccp.tile([P, 1], F32, name=f"cct{b}", tag="cct")
                nc.sync.dma_start(out=cct[:pp], in_=ccv[t0:t0 + pp])
                nc.sync.dma_start(out=tl[:pp], in_=nz[t0:t0 + pp])
                nc.vector.tensor_scalar(out=tl[:pp], in0=tl[:pp],
                                        scalar1=float(na), scalar2=cct[:pp],
                                        op0=Alu.mult, op1=Alu.add)
                nc.sync.dma_start(out=zout[t0:t0 + pp], in_=tl[:pp])

    _fix_sync_waits(nc)
    return nc


# ---------------------------------------------------------------- host orbit
def _host_orbit(scal, x1_0, nsteps):
    """Exact fp32 scalar orbit of (X1, cc) under saturated E=1e6, mirroring
    the device op sequence bit-for-bit (numpy f32 ops == device DVE ops)."""
    f32 = np.float32
    cb, iff, ir, cp, dm = (f32(scal[k]) for k in (
        "coupling_base", "internal_forward", "internal_reverse",
        "center_pull", "damping"))
    c01 = f32(cb * f32(0.1))
    dm1 = f32(f32(1.0) - dm)
    aa = f32(f32(0.005) * SAT_E)
    X1 = f32(x1_0)
    cc = np.empty(nsteps, f32)
    with np.errstate(all="ignore"):
        for t in range(nsteps):
            x1b = f32(f32(X1 + iff) + aa)
            X1 = f32(f32(f32(-cp) * x1b) + x1b)
            s = f32(f32(X1 - ir) + X1)
            phi = f32(f32(0.5) * s)
            yd = f32(f32(phi * dm1) - phi)
            Y = abs(yd)
            p = f32(f32(phi * dm1) * Y)
            # saturated: m2 = fl(2*c01), fl(m2 - c01) == c01 exactly
            cc[t] = f32(c01 + p)
    return cc


# ---------------------------------------------------------------- driver
_nc_cache = {}


def kernel(**inputs):
    zn = np.ascontiguousarray(np.asarray(inputs["z_neighbors"], dtype=np.float32))
    nz = np.ascontiguousarray(np.asarray(inputs["noise"], dtype=np.float32))
    scal = {k: np.float32(inputs[k]) for k in (
        "coupling_base", "internal_forward", "internal_reverse",
        "center_pull", "damping", "noise_amplitude")}
    assert zn.shape == (T, N) and nz.shape == (T, N)
    f32 = np.float32
    na = scal["noise_amplitude"]
    key = tuple(float(scal[k]) for k in sorted(scal))

    if ("head", key) not in _nc_cache:
        _nc_cache[("head", key)] = _build_head(scal)
    nc_head = _nc_cache[("head", key)]

    out = np.empty((T, N), np.float32)
    state = np.zeros((NCORES, 3, P, FDN), np.float32)
    state[:, 2] = f32(0.01)

    def run_head_chunk(t0):
        nsteps = min(TC, T - t0)
        in_maps = []
        for c in range(NCORES):
            nsl = slice(c * NL, (c + 1) * NL)
            znc = zn[t0:t0 + nsteps, nsl]
            nzc = nz[t0:t0 + nsteps, nsl]
            if nsteps < TC:  # pad; padded-step outputs are discarded
                pad = ((0, TC - nsteps), (0, 0))
                znc = np.pad(znc, pad)
                nzc = np.pad(nzc, pad)
            in_maps.append({"zn": znc, "nz": nzc, "state_in": state[c]})
        res = run_bass_kernel_spmd(nc_head, in_maps,
                                   core_ids=list(range(NCORES))).results
        for c in range(NCORES):
            out[t0:t0 + nsteps, c * NL:(c + 1) * NL] = res[c]["zout"][:nsteps]
            state[c] = res[c]["state_out"]

    run_head_chunk(0)

    # ---- host verification of the absorbing state at t = TC ----
    ntail = T - TC
    fast = ntail > 0 and not os.environ.get("DENNIS_FORCE_FALLBACK")
    E_all = state[:, 2].reshape(-1)
    X1_all = state[:, 1].reshape(-1)
    if fast and not np.all(E_all == SAT_E):
        fast = False
    if fast and not np.all(X1_all == X1_all[0]):
        fast = False
    if fast:
        cc = _host_orbit(scal, X1_all[0], ntail)
        # step TC: tension uses the head's per-node Z_{TC-1} directly
        ten0_min = np.abs((zn[TC] - out[TC - 1]).astype(f32)).min()
        if not np.isfinite(cc).all():
            fast = False
        elif ten0_min < 300.0:
            fast = False
        elif ntail > 1:
            # later steps: |Z_{t-1}| >= |cc_{t-1}| - max|na*nz|; tension
            # >= that - max|zn| must stay >= 300 (>> the 200.01 needed to
            # hold the E clip, and >> the 0.01 coupling threshold)
            m_n = float(f32(na)) * float(np.abs(nz[TC:]).max())
            m_z = float(np.abs(zn[TC + 1:]).max())
            if float(np.abs(cc[:-1]).min()) - m_n - m_z < 300.0:
                fast = False

    if fast:
        if ("tail", key, ntail) not in _nc_cache:
            _nc_cache[("tail", key, ntail)] = _build_tail(scal, ntail)
        nc_tail = _nc_cache[("tail", key, ntail)]
        ccin = cc.reshape(ntail, 1)
        in_maps = [{"nz": np.ascontiguousarray(nz[TC:, c * NL:(c + 1) * NL]),
                    "cc": ccin} for c in range(NCORES)]
        res = run_bass_kernel_spmd(nc_tail, in_maps,
                                   core_ids=list(range(NCORES))).results
        for c in range(NCORES):
            out[TC:, c * NL:(c + 1) * NL] = res[c]["zout"]
    else:
        for t0 in range(TC, T, TC):
            run_head_chunk(t0)

    return out


if __name__ == "__main__":
    rng = np.random.default_rng(0)
    demo = {
        "z_neighbors": rng.standard_normal((T, N), dtype=np.float32) * 0.1,
        "noise": rng.standard_normal((T, N), dtype=np.float32),
        "coupling_base": np.float32(0.05),
        "internal_forward": np.float32(0.02),
        "internal_reverse": np.float32(0.01),
        "center_pull": np.float32(0.3),
        "damping": np.float32(0.01),
        "noise_amplitude": np.float32(0.001),
    }
    o = kernel(**demo)
    print("kernel ran:", o.shape, o.dtype, np.abs(o).max())
